# revision 12
# baseline (speedup 1.0000x reference)
"""Multi-head self-attention (B=2, S=2048, E=1024, H=16, causal) on 8 trn2 cores.

v3: warm-start + lean-PE schedule (~159us, from 193-218us for v2).
Core c handles batch c//4, heads [4*(c%4), 4*(c%4)+4), window-major as
v2 (512-query windows, head pairs at partitions 0-63/64-127, K=64 score
matmuls row-group-paired, QKV/out-proj as filler in the exp-wait slack),
plus:

- inputs packed p-major on host; 8 large HWDGE DMAs (qt per-g on sync,
  weights on scalar) so the first QKV matmul lands ~6us in and the HAM
  clock gate warms once and stays warm (warmup matmuls cover the load).
- no K=1 bias matmuls: v/out biases are folded on the host
  (out += bv @ Wout + bout is exact), q/k biases compile in only when
  nonzero (they are zero for this problem's inputs).
- compact diagonal score tiles: the two heads' valid column ranges are
  packed [0:wdt]+[wdt:2*wdt] so exp covers no masked hole and needs no
  memset.
- softmax denominators: ones column in V as v2, but 1/l is produced with
  reciprocal_approx_fast on the l-row and broadcast across partitions
  with gpsimd partition_broadcast -- no DRAM bounce, no sync-queue DMA
  chain.
- output projection runs as late filler (deferred ~2 windows) and y is
  written from the sync queue.
"""

import os
from collections import deque
from contextlib import ExitStack

import ml_dtypes
import numpy as np

import concourse.bass as bass
import concourse.mybir as mybir
import concourse.tile as tile
from concourse import bacc, library_config
from concourse.bass_utils import run_bass_kernel_spmd

f32 = mybir.dt.float32
bf16 = mybir.dt.bfloat16
bfnp = ml_dtypes.bfloat16

S = 2048
E = 1024
HC = 4  # heads per core
D = 64
C = HC * D  # 256 per-core head dims
NE = E // 128  # 8 contraction chunks
NW = 4  # 512-query windows
WARMUP = 34

Exp = mybir.ActivationFunctionType.Exp


NORM_MODE = "new"  # "bounce" keeps the old DRAM-bounce normalization for debugging


def _build_kernel(tc, qt, wq, wk, wv, wo, mk, bq, bk, y, with_bias):
    nc = tc.nc
    rrow = None
    if NORM_MODE == "bounce":
        rrow = nc.dram_tensor("rrow", [16, 512], f32).ap()
    with ExitStack() as ctx:
        const = ctx.enter_context(tc.tile_pool(name="const", bufs=1))
        qt_all = const.tile([128, 4, NE, 512], bf16)
        wq_sb = const.tile([128, NE, C], bf16)
        wk_sb = const.tile([128, NE, C], bf16)
        wv_sb = const.tile([128, NE, C], bf16)
        wo_sb = const.tile([128, 2, E], bf16)
        ones_sb = const.tile([1, 512], bf16)
        qT_sb = const.tile([128, 2, S], bf16)
        kT_sb = const.tile([128, 2, S], bf16)
        v_sb = [
            const.tile([128, HC, D + 1], bf16, tag=f"v{si}", name=f"v_sb{si}")
            for si in range(16)
        ]
        out_sb = const.tile([128, 2, S], bf16)
        if with_bias:
            bq_sb = const.tile([1, C], bf16)
            bk_sb = const.tile([1, C], bf16)
        lp = ctx.enter_context(tc.tile_pool(name="lp", bufs=2))
        ptp = ctx.enter_context(tc.tile_pool(name="ptp", bufs=8))
        yp = ctx.enter_context(tc.tile_pool(name="yp", bufs=3))
        att = ctx.enter_context(tc.tile_pool(name="att", bufs=2, space="PSUM"))

        # --- preamble: proxy gpsimd library (tensor_tensor +
        # partition_broadcast together -- avoids mid-kernel ucode
        # reload ping-pong), then input DMAs, memsets, exp-table ---
        nc.gpsimd.load_library(library_config.proxy)
        qt_r = qt.rearrange("(p g x) -> p g x", p=128, g=4)
        for g in range(4):
            src_g = qt_r[:, g].rearrange("p (i s) -> p i s", i=NE)
            if g < 2:
                h = NE // 2
                nc.sync.dma_start(qt_all[:, g, 0:h], src_g[:, 0:h])
                nc.sync.dma_start(qt_all[:, g, h:NE], src_g[:, h:NE])
            else:
                nc.sync.dma_start(qt_all[:, g], src_g)
        for wsrc, wdst in ((wq, wq_sb), (wk, wk_sb), (wv, wv_sb)):
            nc.scalar.dma_start(
                wdst[:],
                wsrc.rearrange("(p x) -> p x", p=128).rearrange(
                    "p (i c) -> p i c", i=NE
                ),
            )
        nc.scalar.dma_start(
            wo_sb[:],
            wo.rearrange("(p x) -> p x", p=128).rearrange("p (m e) -> p m e", m=2),
        )
        mk_sb = const.tile([128, 128], bf16)
        nc.scalar.dma_start(mk_sb[:], mk.rearrange("(p c) -> p c", p=128))
        if with_bias:
            nc.scalar.dma_start(bq_sb[:], bq[:])
            nc.scalar.dma_start(bk_sb[:], bk[:])

        ones2 = const.tile([128, 512], bf16)
        nc.vector.memset(ones2[:], 1.0)
        nc.vector.memset(ones_sb[:], 1.0)
        for si in range(16):
            nc.gpsimd.memset(v_sb[si][:, :, D : D + 1], 1.0)
        twarm = const.tile([1, 1], f32)
        nc.scalar.activation(twarm[:], ones_sb[0:1, 0:1], Exp)

        # --- filler generators (~2 PE ops per pump) ---
        def gen_warm():
            # full-K matmuls: K=1 warmups do not register as PE activity
            # for the HAM clock gate
            wt = att.tile([128, 512], f32, tag="fill", bufs=2, name="warm")
            for r in range(WARMUP):
                nc.tensor.matmul(
                    wt[:],
                    lhsT=ones2[:, 0:128],
                    rhs=ones2[:, 0:512],
                    start=True,
                    stop=True,
                )
            return
            yield

        def gen_qk(m, g, use_scalar):
            for wsb, dst, bsb, nm in (
                (wq_sb, qT_sb, "bq", "q"),
                (wk_sb, kT_sb, "bk", "k"),
            ):
                ps = att.tile(
                    [128, 512], f32, tag="fill", bufs=2, name=f"p{nm}{m}{g}"
                )
                for i in range(NE):
                    nc.tensor.matmul(
                        ps[:],
                        lhsT=wsb[:, i, 128 * m : 128 * m + 128],
                        rhs=qt_all[:, g, i, :],
                        start=(i == 0),
                        stop=(i == NE - 1) if not with_bias else False,
                    )
                    if i % 2 == 1:
                        yield
                if with_bias:
                    bt = bq_sb if bsb == "bq" else bk_sb
                    nc.tensor.matmul(
                        ps[:],
                        lhsT=bt[0:1, 128 * m : 128 * m + 128],
                        rhs=ones_sb[0:1, 0:512],
                        start=False,
                        stop=True,
                    )
                if use_scalar:
                    nc.scalar.copy(dst[:, m, 512 * g : 512 * g + 512], ps[:])
                else:
                    nc.vector.tensor_copy(
                        dst[:, m, 512 * g : 512 * g + 512], ps[:]
                    )
                yield

        def gen_v(si):
            ps = att.tile([128, 512], f32, tag="fill", bufs=2, name=f"pv{si}")
            sg, so = si // 4, si % 4
            for i in range(NE):
                nc.tensor.matmul(
                    ps[:, 0:C],
                    lhsT=qt_all[:, sg, i, 128 * so : 128 * so + 128],
                    rhs=wv_sb[:, i, :],
                    start=(i == 0),
                    stop=(i == NE - 1),
                )
                if i % 2 == 1:
                    yield
            nc.vector.tensor_copy(
                v_sb[si][:, :, 0:D],
                ps[:, 0:C].rearrange("p (h d) -> p h d", h=HC),
            )
            yield

        def gen_out(t):
            ysb = yp.tile([128, E], bf16, tag="y", bufs=3, name=f"ysb{t}")
            for e in range(2):
                if t >= 12:
                    ps = att.tile(
                        [128, 1024], f32, tag="sc", bufs=2, name=f"py{t}_{e}"
                    )
                    ps = ps[:, 0:512]
                else:
                    ps = att.tile(
                        [128, 512], f32, tag="fill", bufs=2, name=f"py{t}_{e}"
                    )
                for m in range(2):
                    nc.tensor.matmul(
                        ps[:],
                        lhsT=out_sb[:, m, 128 * t : 128 * t + 128],
                        rhs=wo_sb[:, m, 512 * e : 512 * e + 512],
                        start=(m == 0),
                        stop=(m == 1),
                    )
                yield
                if e == 0:
                    nc.scalar.copy(ysb[:, 512 * e : 512 * e + 512], ps[:])
                else:
                    nc.vector.tensor_copy(
                        ysb[:, 512 * e : 512 * e + 512], ps[:]
                    )
                yield
            nc.sync.dma_start(y[t, :, :], ysb[:])

        mainq = deque()
        outq = deque()

        def pump(n=1):
            k = 0
            while k < n:
                q = None
                if mainq and not isinstance(mainq[0], str):
                    q = mainq
                elif outq:
                    q = outq
                else:
                    return
                try:
                    next(q[0])
                    k += 1
                except StopIteration:
                    q.popleft()

        def drain_to(marker):
            while mainq:
                if isinstance(mainq[0], str):
                    mk = mainq.popleft()
                    if mk == marker:
                        return
                    continue
                try:
                    next(mainq[0])
                except StopIteration:
                    mainq.popleft()

        def drain_all():
            while mainq or outq:
                q = mainq if mainq else outq
                if isinstance(q[0], str):
                    q.popleft()
                    continue
                try:
                    next(q[0])
                except StopIteration:
                    q.popleft()

        mainq.append(gen_warm())
        mainq.append(gen_qk(0, 0, True))
        mainq.append(gen_qk(1, 0, True))
        for si in range(4):
            mainq.append(gen_v(si))
        mainq.append("w0")
        for g in range(1, 4):
            mainq.append(gen_qk(0, g, False))
            mainq.append(gen_qk(1, g, False))
            for si in range(4 * g, 4 * g + 4):
                mainq.append(gen_v(si))
            mainq.append(f"w{g}")

        # --- attention: window-major, pair-interleaved, software-pipelined ---
        def issue_scores(w, pair, kc):
            # heads packed against the 512 (PSUM-bank) boundary: j0 at
            # [e0:512], j1 at [512:512+wdt] -- exp covers the contiguous
            # hole-free range [e0 : 1024-e0]; each matmul stays in one bank.
            e0 = 128 * (kc - 4 * w) if kc >= 4 * w else 0
            wdt = 512 - e0
            psj = att.tile(
                [128, 1024], f32, tag="sc", bufs=2, name=f"sc{w}_{pair}_{kc}"
            )
            for j in (0, 1):
                b0 = 64 * j
                nc.tensor.matmul(
                    psj[:, e0 + j * wdt : 512 + j * wdt],
                    lhsT=kT_sb[b0 : b0 + 64, pair, 128 * kc : 128 * kc + 128],
                    rhs=qT_sb[
                        b0 : b0 + 64, pair, 512 * w + e0 : 512 * w + 512
                    ],
                    start=True,
                    stop=True,
                )
            return psj, e0

        pending_muls = []
        ps_fin = []
        presc = {}
        # window w's output projections become fillers at out_sched[w]
        out_sched = {0: (2, 0), 1: (3, 0), 2: (3, 1)}

        def flush_muls():
            for fn in pending_muls:
                fn()
            pending_muls.clear()

        for w in range(NW):
            drain_to(f"w{w}")
            nkc = 4 * w + 4
            for pair in range(2):
                last_pair = w == NW - 1 and pair == 1
                avj = [
                    att.tile(
                        [D + 1, 512],
                        f32,
                        tag="av",
                        bufs=2,
                        name=f"av{w}_{pair}_{j}",
                    )
                    for j in (0, 1)
                ]
                pre_pt = presc.pop((w, pair), None)
                if pre_pt is None:
                    sc_cur = issue_scores(w, pair, 0)
                for kc in range(nkc):
                    if kc == 2:
                        flush_muls()
                        for wd, sched in out_sched.items():
                            if sched == (w, pair):
                                for t in range(4 * wd, 4 * wd + 4):
                                    outq.append(gen_out(t))
                    if pre_pt is not None and kc < len(pre_pt):
                        # scores+exp for this kc were issued at the end of
                        # the previous pair; AV can run immediately
                        if kc == len(pre_pt) - 1:
                            sc_cur = issue_scores(w, pair, kc + 1)
                        pump(3)
                        for j in (0, 1):
                            nc.tensor.matmul(
                                avj[j][:, 0:512],
                                lhsT=v_sb[kc][:, 2 * pair + j, :],
                                rhs=pre_pt[kc][:, 512 * j : 512 * j + 512],
                                start=(kc == 0),
                                stop=False,
                                skip_group_check=True,
                            )
                        continue
                    psj, e0 = sc_cur
                    wdt = 512 - e0
                    pt = ptp.tile(
                        [128, 1024], bf16, tag="pt", name=f"pt{w}_{pair}_{kc}"
                    )
                    nc.scalar.activation(
                        pt[:, e0 : 1024 - e0], psj[:, e0 : 1024 - e0], Exp
                    )
                    if kc >= 4 * w:
                        # zero the in-block upper triangle (keys > query)
                        # with a DVE multiply by the precomputed triu mask
                        # (affine_select on gpsimd would force ucode
                        # library reloads against partition_broadcast)
                        for j in (0, 1):
                            blk = pt[:, e0 + j * wdt : e0 + j * wdt + 128]
                            nc.vector.tensor_mul(blk, blk, mk_sb[:])
                    if kc + 1 < nkc:
                        sc_cur = issue_scores(w, pair, kc + 1)
                    if last_pair and kc >= nkc - 2:
                        pass  # prefetched projections fill the PE here
                    else:
                        pump(3)
                    if last_pair and kc == nkc - 1:
                        wt2 = att.tile(
                            [128, 1024], f32, tag="sc", bufs=2,
                            name="warmtail",
                        )
                        for r in range(8):
                            nc.tensor.matmul(
                                wt2[:, 0:512],
                                lhsT=ones2[:, 0:128],
                                rhs=ones2[:, 0:512],
                                start=True,
                                stop=True,
                                skip_group_check=True,
                            )
                        for t in (12, 13):
                            ps = att.tile(
                                [128, 1024], f32, tag="sc", bufs=2,
                                name=f"pf{t}",
                            )
                            for e in (0, 1):
                                nc.tensor.matmul(
                                    ps[:, 512 * e : 512 * e + 512],
                                    lhsT=out_sb[:, 0, 128 * t : 128 * t + 128],
                                    rhs=wo_sb[:, 0, 512 * e : 512 * e + 512],
                                    start=True,
                                    stop=False,
                                    skip_group_check=True,
                                )
                            ps_fin.append(ps)
                        for t in (14, 15):
                            psa = att.tile(
                                [128, 512], f32, tag="fill", bufs=2,
                                name=f"pfa{t}_0",
                            )
                            psb = att.tile(
                                [128, 512], f32, tag="fill", bufs=2,
                                name=f"pfa{t}_1",
                            )
                            for e, pse in ((0, psa), (1, psb)):
                                nc.tensor.matmul(
                                    pse[:],
                                    lhsT=out_sb[:, 0, 128 * t : 128 * t + 128],
                                    rhs=wo_sb[:, 0, 512 * e : 512 * e + 512],
                                    start=True,
                                    stop=False,
                                    skip_group_check=True,
                                )
                            ps_fin.append((psa, psb))
                    for j in (0, 1):
                        nc.tensor.matmul(
                            avj[j][:, e0:512],
                            lhsT=v_sb[kc][:, 2 * pair + j, :],
                            rhs=pt[:, e0 + j * wdt : 512 + j * wdt],
                            start=(kc == 0),
                            stop=(kc == nkc - 1),
                            skip_group_check=True,
                        )
                # denominators + normalize into out_sb: stage av to SBUF,
                # 1/l via approx-recip on the l row, partition_broadcast
                # across the 64 dim-partitions (no DRAM bounce), then the
                # final muls are deferred past the next pair's start.
                if pair == 0:
                    psj0, _ = issue_scores(w, 1, 0)
                    pt0 = ptp.tile(
                        [128, 1024], bf16, tag="pt", name=f"ptpre{w}"
                    )
                    nc.scalar.activation(pt0[:, 0:1024], psj0[:, 0:1024], Exp)
                    if 0 >= 4 * w:  # w0: kc0 is diagonal, mask it
                        for j in (0, 1):
                            blk = pt0[:, 512 * j : 512 * j + 128]
                            nc.vector.tensor_mul(blk, blk, mk_sb[:])
                    pre = [pt0]
                    if w >= 1:  # kc1 is full-width for w>=1: 2-deep entry
                        psj1, _ = issue_scores(w, 1, 1)
                        pt1 = ptp.tile(
                            [128, 1024], bf16, tag="pt", name=f"ptpre{w}b"
                        )
                        nc.scalar.activation(
                            pt1[:, 0:1024], psj1[:, 0:1024], Exp
                        )
                        pre.append(pt1)
                    presc[(w, 1)] = pre
                avs_j, rb_j = [], []
                if NORM_MODE == "new":
                    # l-row to partition 0 (DVE copy from PSUM), 1/l in
                    # place, then gpsimd partition_broadcast (partition-0
                    # src required).  Emitted before the avs copies so the
                    # broadcast unblocks as early as possible.
                    for j in (0, 1):
                        lrec = lp.tile(
                            [1, 512], f32, tag="lrec", bufs=4,
                            name=f"lr{w}{pair}{j}",
                        )
                        nc.vector.tensor_copy(lrec[:], avj[j][D : D + 1, :])
                        nc.vector.reciprocal_approx_fast(
                            out=lrec[:], in_=lrec[:]
                        )
                        rb = lp.tile(
                            [64, 512], f32, tag="rb", bufs=4,
                            name=f"rb{w}{pair}{j}",
                        )
                        nc.gpsimd.partition_broadcast(
                            rb[:], lrec[0:1, :], channels=64
                        )
                        rb_j.append(rb)
                if last_pair:
                    # nothing follows: mul straight from PSUM, skip staging
                    avs_j = avj
                else:
                    for j in (0, 1):
                        avs = lp.tile(
                            [D + 1, 512], f32, tag="avs", bufs=4,
                            name=f"as{w}{pair}{j}",
                        )
                        nc.vector.tensor_copy(avs[:], avj[j][:, :])
                        avs_j.append(avs)
                for j in (0, 1):
                    if NORM_MODE == "new":
                        break
                    lt = lp.tile(
                        [128, 4], f32, tag="lt", bufs=4, name=f"lt{w}{pair}{j}"
                    )
                    l_row = avs_j[j][D : D + 1, :]
                    nc.sync.dma_start(
                        lt[:],
                        bass.AP(
                            tensor=l_row.tensor,
                            offset=l_row.offset,
                            ap=[list(l_row.ap[0]), [4, 128], [1, 4]],
                        ),
                    )
                    nc.vector.reciprocal(lt[:], lt[:])
                    ridx = (2 * pair + j) * 4 + w
                    nc.sync.dma_start(
                        rrow[ridx, :].rearrange("(p c) -> p c", p=128), lt[:]
                    )
                    rb = lp.tile(
                        [64, 512], f32, tag="rb", bufs=4, name=f"rb{w}{pair}{j}"
                    )
                    rr = rrow[ridx, :]
                    nc.sync.dma_start(
                        rb[:],
                        bass.AP(
                            tensor=rr.tensor,
                            offset=rr.offset,
                            ap=[[0, 64], [1, 512]],
                        ),
                    )
                    rb_j.append(rb)

                def mk_mul(w=w, pair=pair, avs_j=avs_j, rb_j=rb_j):
                    for j in (0, 1):
                        nc.vector.tensor_mul(
                            out_sb[
                                64 * j : 64 * j + 64,
                                pair,
                                512 * w : 512 * w + 512,
                            ],
                            avs_j[j][0:D, :],
                            rb_j[j][:],
                        )

                pending_muls.append(mk_mul)
        flush_muls()
        for idx, t in enumerate((12, 13, 14, 15)):
            ps = ps_fin[idx]
            if t < 14:
                pse0, pse1 = ps[:, 0:512], ps[:, 512:1024]
            else:
                pse0, pse1 = ps[0][:], ps[1][:]
            ysb = yp.tile([128, E], bf16, tag="y", bufs=3, name=f"ysbf{t}")
            for e, pse in ((0, pse0), (1, pse1)):
                nc.tensor.matmul(
                    pse,
                    lhsT=out_sb[:, 1, 128 * t : 128 * t + 128],
                    rhs=wo_sb[:, 1, 512 * e : 512 * e + 512],
                    start=False,
                    stop=True,
                    skip_group_check=True,
                )
            eng = nc.scalar if t % 2 else nc.sync
            nc.scalar.copy(ysb[:, 0:512], pse0)
            eng.dma_start(y[t, :, 0:512], ysb[:, 0:512])
            nc.vector.tensor_copy(ysb[:, 512:1024], pse1)
            eng.dma_start(y[t, :, 512:1024], ysb[:, 512:1024])
        drain_all()


_NC = {}


def build_nc(with_bias):
    if with_bias in _NC:
        return _NC[with_bias]
    nc = bacc.Bacc("TRN2", target_bir_lowering=False, debug=False, num_devices=8)
    qt = nc.dram_tensor("qt", [128 * 4 * NE * 512], bf16, kind="ExternalInput").ap()
    wq = nc.dram_tensor("wq", [128 * NE * C], bf16, kind="ExternalInput").ap()
    wk = nc.dram_tensor("wk", [128 * NE * C], bf16, kind="ExternalInput").ap()
    wv = nc.dram_tensor("wv", [128 * NE * C], bf16, kind="ExternalInput").ap()
    wo = nc.dram_tensor("wo", [128 * 2 * E], bf16, kind="ExternalInput").ap()
    mk = nc.dram_tensor("mk", [128 * 128], bf16, kind="ExternalInput").ap()
    bq = bk = None
    if with_bias:
        bq = nc.dram_tensor("bq", [1, C], bf16, kind="ExternalInput").ap()
        bk = nc.dram_tensor("bk", [1, C], bf16, kind="ExternalInput").ap()
    y = nc.dram_tensor("y", [16, 128, E], bf16, kind="ExternalOutput").ap()
    with tile.TileContext(nc) as tc:
        _build_kernel(tc, qt, wq, wk, wv, wo, mk, bq, bk, y, with_bias)
    nc.compile()
    _NC[with_bias] = nc
    return nc


def make_in_maps(Q, Wqkv, bqkv, with_bias):
    """Per-core input dicts (8 cores: batch-major, then head-group).

    All packs are p-major so every DMA reads DRAM fully contiguously.
    """
    in_maps = []
    for c in range(8):
        b, hq = c // 4, c % 4
        cs = C * hq
        # qt[p, g, i, s] = Q[b][512g+s, 128i+p]
        qt_np = (
            np.ascontiguousarray(
                Q[b].reshape(4, 512, NE, 128).transpose(3, 0, 2, 1)
            )
            .astype(bfnp)
            .reshape(-1)
        )

        def packw(w):
            # [E, C] -> [p, i, c] p-major
            return (
                np.ascontiguousarray(
                    w.reshape(NE, 128, -1).transpose(1, 0, 2)
                )
                .astype(bfnp)
                .reshape(-1)
            )

        m = {
            "qt": qt_np,
            "mk": TRIU_MASK,
            "wq": packw(Wqkv[:, cs : cs + C] * 0.125),
            "wk": packw(Wqkv[:, E + cs : E + cs + C]),
            "wv": packw(Wqkv[:, 2 * E + cs : 2 * E + cs + C]),
            "wo": (
                np.ascontiguousarray(
                    Wout_cache[0][cs : cs + C, :]
                    .reshape(2, 128, E)
                    .transpose(1, 0, 2)
                )
                .astype(bfnp)
                .reshape(-1)
            ),
        }
        if with_bias:
            m["bq"] = (bqkv[cs : cs + C] * 0.125).reshape(1, C).astype(bfnp)
            m["bk"] = bqkv[E + cs : E + cs + C].reshape(1, C).astype(bfnp)
        in_maps.append(m)
    return in_maps


Wout_cache = [None]
TRIU_MASK = np.triu(np.ones((128, 128), dtype=np.float32)).astype(bfnp).reshape(-1)


def kernel(Q, Wqkv, bqkv, Wout, bout, _trace=False, _trace_kwargs=None):
    Q = np.asarray(Q, dtype=np.float32)
    Wqkv = np.asarray(Wqkv, dtype=np.float32)
    bqkv = np.asarray(bqkv, dtype=np.float32)
    Wout = np.asarray(Wout, dtype=np.float32)
    bout = np.asarray(bout, dtype=np.float32)
    Wout_cache[0] = Wout

    # q/k biases need in-kernel handling; v bias folds into the output
    # on the host exactly: out += bqkv[2E:] @ Wout + bout.
    with_bias = bool(np.any(bqkv[: 2 * E]))
    nc = build_nc(with_bias)
    in_maps = make_in_maps(Q, Wqkv, bqkv, with_bias)

    kwargs = {}
    if _trace:
        kwargs = dict(trace=True, trace_cores=list(range(8)))
        if _trace_kwargs:
            kwargs.update(_trace_kwargs)
    res = run_bass_kernel_spmd(nc, in_maps, core_ids=list(range(8)), **kwargs)

    out = np.zeros((2, S, E), dtype=np.float32)
    for c in range(8):
        yc = np.asarray(res.results[c]["y"]).astype(np.float32).reshape(S, E)
        out[c // 4] += yc
    out += (bout + bqkv[2 * E :].astype(np.float32) @ Wout)[None, None, :]
    if _trace:
        kernel._last_results = res
    return out
